# revision 7
# baseline (speedup 1.0000x reference)
"""Trainium2 Bass kernel for nn_LlamaAttention_16372415332980.

Llama GQA attention (B=1, S=2048, D=2048, NH=32, NKV=8, HD=64) with RoPE and
causal mask, tensor-parallel over 8 NeuronCores: core c owns kv-head c
(= query heads 4c..4c+3). Per-core pipeline, all matmuls fp32r:

  1. projections: qT/kT/vT = (w.T @ xT-chunks), x transposed host-side
  2. RoPE: 2 DVE muls (cos/sin) + one PE "select+swap" combine matmul per head
     (weight columns host-permuted so even/odd lanes form 32-row blocks)
  3. scoresT[kk,sq] = kT.T @ qT per 128x512 block, causal blocks skipped,
     mask applied by accumulating a -30000 bias tile via identity matmul
  4. exp on ScalarE (PSUM->SBUF); A@V with a ones-row appended to V so the
     softmax denominator comes out of the same matmul (row 64 of ctx PSUM)
  5. normalize, AllToAll the per-head context (seq-sharding it), and
  6. output projection d-outer over 8 PSUM accumulators with row-permuted wo;
     each core emits output rows [256c, 256c+256); host concatenates.
"""

import math
import sys

for _p in ("/opt/trn_rl_repo",):
    if _p not in sys.path:
        sys.path.insert(0, _p)

import numpy as np

import concourse.bacc as bacc
import concourse.bass as bass
import concourse.mybir as mybir
import concourse.tile as tile
from concourse.bass_utils import run_bass_kernel_spmd

B, S, DIM = 1, 2048, 2048
NH, NKV, HD = 32, 8, 64
NREP = NH // NKV
N_CORES = 8
SCALE = 1.0 / math.sqrt(HD)
ROPE_BASE = 10000.0

F32 = mybir.dt.float32
F32R = mybir.dt.float32r

NKK = S // 128      # 16 key tiles of 128
NSQ = S // 512      # 4 query chunks of 512
NKCH = DIM // 128   # 16 contraction chunks
SKIP, FULL = -1, -2
NEG = -30000.0


def _analyze_mask(mask):
    """Classify each [128 kk, 512 sq] block of the (head-shared) mask.

    Returns (state[t][c], patterns): state is SKIP / FULL / pattern index;
    patterns[p] is a [128, 512] float32 0/-30000 bias tile.
    scoresT[kk, sq] is masked where mask[0,0,sq,kk] == 0.
    """
    m = np.asarray(mask).reshape(S, S)
    patterns = []
    index = {}
    state = [[SKIP] * NSQ for _ in range(NKK)]
    for t in range(NKK):
        for c in range(NSQ):
            blk = m[c * 512:(c + 1) * 512, t * 128:(t + 1) * 128].T  # [128,512]
            if not blk.any():
                state[t][c] = SKIP
            elif blk.all():
                state[t][c] = FULL
            else:
                key = blk.tobytes()
                if key not in index:
                    index[key] = len(patterns)
                    patterns.append((1.0 - blk.astype(np.float32)) * NEG)
                state[t][c] = index[key]
    return state, patterns


def _build(state, npat):
    """Build + trace the SPMD Tile program for a given mask block structure."""
    nc = bacc.Bacc("TRN2", target_bir_lowering=False, debug=False,
                   num_devices=N_CORES)

    # ---- external inputs (per-core, host-prepped) ----
    xT_e = nc.dram_tensor("xT", [DIM, S], F32R, kind="ExternalInput")
    wq_e = nc.dram_tensor("wq", [DIM, NREP * HD], F32R, kind="ExternalInput")
    wkv_e = nc.dram_tensor("wkv", [DIM, 2 * HD], F32R, kind="ExternalInput")
    wo_e = nc.dram_tensor("wo", [DIM, DIM], F32R, kind="ExternalInput")
    cos_e = nc.dram_tensor("cosx", [128, S], F32, kind="ExternalInput")
    sin_e = nc.dram_tensor("sinx", [128, S], F32, kind="ExternalInput")
    selA_e = nc.dram_tensor("selA", [128, 128], F32R, kind="ExternalInput")
    selB_e = nc.dram_tensor("selB", [128, 128], F32R, kind="ExternalInput")
    ident_e = nc.dram_tensor("ident", [128, 128], F32R, kind="ExternalInput")
    onesr_e = nc.dram_tensor("onesr", [128, 64], F32R, kind="ExternalInput")
    mb_e = nc.dram_tensor("maskbias", [128, max(npat, 1) * 512], F32R,
                          kind="ExternalInput")
    out_e = nc.dram_tensor("out", [S // N_CORES, DIM], F32, kind="ExternalOutput")

    rg = [list(range(N_CORES))]

    with tile.TileContext(nc) as tc:
        with (
            tc.tile_pool(name="const", bufs=1) as constp,
            tc.tile_pool(name="wqp", bufs=1) as wqp,
            tc.tile_pool(name="xsp", bufs=2) as xsp,
            tc.tile_pool(name="qkp", bufs=1) as qkp,
            tc.tile_pool(name="ropep", bufs=2) as ropep,
            tc.tile_pool(name="expp", bufs=2) as expp,
            tc.tile_pool(name="ctxp", bufs=2) as ctxp,
            tc.tile_pool(name="workp", bufs=2) as workp,
            tc.tile_pool(name="wop", bufs=3) as wop,
            tc.tile_pool(name="ctxall", bufs=1) as ctxallp,
            tc.tile_pool(name="psA", bufs=4, space="PSUM") as psA,   # 4 banks
            tc.tile_pool(name="psS", bufs=1, space="PSUM") as psS,   # 2 banks
            tc.tile_pool(name="psX", bufs=2, space="PSUM") as psX,   # 2 banks
            tc.tile_pool(name="dram", bufs=1, space="DRAM") as dram,
        ):
            # ---- constants / weights to SBUF ----
            cos_s = constp.tile([128, S], F32, tag="cos")
            sin_s = constp.tile([128, S], F32, tag="sin")
            selA = constp.tile([128, 128], F32R, tag="selA")
            selB = constp.tile([128, 128], F32R, tag="selB")
            ident = constp.tile([128, 128], F32R, tag="ident")
            onesr = constp.tile([128, 64], F32R, tag="onesr")
            mb_s = constp.tile([128, max(npat, 1) * 512], F32R, tag="mb")
            nc.sync.dma_start(cos_s[:], cos_e[:])
            nc.sync.dma_start(sin_s[:], sin_e[:])
            nc.sync.dma_start(selA[:], selA_e[:])
            nc.sync.dma_start(selB[:], selB_e[:])
            nc.sync.dma_start(ident[:], ident_e[:])
            nc.sync.dma_start(onesr[:], onesr_e[:])
            nc.sync.dma_start(mb_s[:], mb_e[:])

            wq_s = wqp.tile([128, NKCH, 256], F32R, tag="wq")
            wkv_s = wqp.tile([128, NKCH, 128], F32R, tag="wkv")
            nc.sync.dma_start(
                wq_s[:], wq_e[:].rearrange("(a p) n -> p a n", p=128))
            nc.sync.dma_start(
                wkv_s[:], wkv_e[:].rearrange("(a p) n -> p a n", p=128))

            # ---- persistent activation tiles ----
            qT = [qkp.tile([64, S], F32R, tag=f"qT{h}", name=f"qT{h}")
                  for h in range(NREP)]
            kT = qkp.tile([64, S], F32R, tag="kT")
            v_aug = qkp.tile([128, NKK, 65], F32R, tag="vaug")
            nc.vector.memset(v_aug[:, :, 64].bitcast(F32), 1.0)

            # ================= phase 1: projections + rope =================
            # groups: 0 -> q heads 0,1 ; 1 -> q heads 2,3 ; 2 -> [k | v]
            for c in range(NSQ):
                xs = []
                for kq in range(4):  # contraction quarters (4 k-chunks each)
                    x_t = xsp.tile([128, 4, 512], F32R, tag="xs")
                    nc.sync.dma_start(
                        x_t[:],
                        xT_e[512 * kq:512 * (kq + 1),
                             512 * c:512 * (c + 1)].rearrange(
                                 "(a p) s -> p a s", p=128))
                    xs.append(x_t)
                for g in range(3):
                    acc = psA.tile([128, 512], F32, tag="acc")
                    for kq in range(4):
                        for a in range(4):
                            k = 4 * kq + a
                            w_ap = (wq_s[:, k, 128 * g:128 * (g + 1)]
                                    if g < 2 else wkv_s[:, k, :])
                            nc.tensor.matmul(
                                acc[:], w_ap, xs[kq][:, a, :],
                                start=(k == 0), stop=(k == NKCH - 1))
                    # rope: T1 = acc*cos, T2 = acc*sin (both [128,512])
                    t1 = ropep.tile([128, 512], F32R, tag="t1")
                    t2 = ropep.tile([128, 512], F32R, tag="t2")
                    cs = slice(512 * c, 512 * (c + 1))
                    if g < 2:
                        nc.vector.tensor_tensor(
                            t1[:], acc[:], cos_s[:, cs], mybir.AluOpType.mult)
                        nc.vector.tensor_tensor(
                            t2[:], acc[:], sin_s[:, cs], mybir.AluOpType.mult)
                        for hh in range(2):
                            h = 2 * g + hh
                            rp = psX.tile([64, 512], F32, tag="rope")
                            nc.tensor.matmul(
                                rp[:], selA[:, 64 * hh:64 * hh + 64], t1[:],
                                start=True, stop=False)
                            nc.tensor.matmul(
                                rp[:], selB[:, 64 * hh:64 * hh + 64], t2[:],
                                start=False, stop=True)
                            nc.scalar.copy(qT[h][:, cs], rp[:])
                    else:
                        nc.vector.tensor_tensor(
                            t1[0:64, :], acc[0:64, :], cos_s[0:64, cs],
                            mybir.AluOpType.mult)
                        nc.vector.tensor_tensor(
                            t2[0:64, :], acc[0:64, :], sin_s[0:64, cs],
                            mybir.AluOpType.mult)
                        rp = psX.tile([64, 512], F32, tag="rope")
                        nc.tensor.matmul(rp[:], selA[0:64, 0:64], t1[0:64, :],
                                         start=True, stop=False)
                        nc.tensor.matmul(rp[:], selB[0:64, 0:64], t2[0:64, :],
                                         start=False, stop=True)
                        nc.scalar.copy(kT[:, cs], rp[:])
                        # v: rows 64:127 of acc -> transpose into v_aug
                        vt = workp.tile([128, 512], F32R, tag="vtmp")
                        nc.vector.tensor_copy(vt[64:128, :], acc[64:128, :])
                        for u in range(4):
                            tp = psX.tile([128, 64], F32R, tag="rope")
                            nc.tensor.transpose(
                                tp[:], vt[64:128, 128 * u:128 * (u + 1)],
                                ident[64:128, 64:128])
                            nc.vector.tensor_copy(
                                v_aug[:, 4 * c + u, 0:64], tp[:])

            # ================= phase 2+3: attention per head ===============
            first_t = [min(t for t in range(NKK) if state[t][c] != SKIP)
                       for c in range(NSQ)]
            last_t = [max(t for t in range(NKK) if state[t][c] != SKIP)
                      for c in range(NSQ)]
            a2a_out = []
            for h in range(NREP):
                ctx_ps = [psA.tile([65, 512], F32, tag="acc", name=f"ctx{h}_{c}")
                          for c in range(NSQ)]
                for t in range(NKK):
                    for p in range(2):
                        cs_l = [c for c in (2 * p, 2 * p + 1)
                                if state[t][c] != SKIP]
                        if not cs_l:
                            continue
                        sc = psS.tile([128, 1024], F32, tag="sc")
                        for c in cs_l:
                            o = (c - 2 * p) * 512
                            st = state[t][c]
                            nc.tensor.matmul(
                                sc[:, o:o + 512],
                                kT[:, 128 * t:128 * (t + 1)],
                                qT[h][:, 512 * c:512 * (c + 1)],
                                start=True, stop=(st == FULL))
                            if st != FULL:
                                nc.tensor.matmul(
                                    sc[:, o:o + 512], ident[:],
                                    mb_s[:, 512 * st:512 * (st + 1)],
                                    start=False, stop=True)
                        lo = (cs_l[0] - 2 * p) * 512
                        hi = (cs_l[-1] - 2 * p) * 512 + 512
                        ex = expp.tile([128, 1024], F32R, tag="ex")
                        nc.scalar.activation(
                            ex[:, lo:hi], sc[:, lo:hi],
                            mybir.ActivationFunctionType.Exp)
                        for c in cs_l:
                            o = (c - 2 * p) * 512
                            nc.tensor.matmul(
                                ctx_ps[c][:], v_aug[:, t, :],
                                ex[:, o:o + 512],
                                start=(t == first_t[c]), stop=(t == last_t[c]))
                # normalize + stage for AllToAll
                ctx_sb = ctxp.tile([64, S], F32R, tag="ctxsb")
                for c in range(NSQ):
                    rec = workp.tile([65, 512], F32R, tag="rec")
                    with nc.allow_low_precision(reason="f32r recip feeds matmul"):
                        nc.vector.reciprocal(rec[64:65, :], ctx_ps[c][64:65, :])
                    bc = psX.tile([64, 512], F32, tag="rope")
                    nc.tensor.matmul(bc[:], onesr[64:65, :], rec[64:65, :],
                                     start=True, stop=True)
                    bc_sb = workp.tile([64, 512], F32, tag="bcsb")
                    nc.vector.tensor_copy(bc_sb[:], bc[:])
                    nc.vector.tensor_tensor(
                        ctx_sb[:, 512 * c:512 * (c + 1)],
                        ctx_ps[c][0:64, :], bc_sb[:], mybir.AluOpType.mult)
                a_in = dram.tile([8 * 64, 256], F32R, tag=f"a2ai{h}")
                a_out = dram.tile([8 * 64, 256], F32R, tag=f"a2ao{h}")
                a2a_out.append(a_out)
                nc.sync.dma_start(
                    a_in[:].rearrange("(j d) s -> d j s", j=8),
                    ctx_sb[:].rearrange("d (j s) -> d j s", j=8))
                nc.gpsimd.collective_compute(
                    "AllToAll", mybir.AluOpType.bypass, replica_groups=rg,
                    ins=[a_in.opt()], outs=[a_out.opt()])

            # ================= phase 4: output projection ==================
            ctx_all = ctxallp.tile([128, NKCH, 256], F32R, tag="call")
            for j in range(NREP):
                nc.sync.dma_start(
                    ctx_all[:, 4 * j:4 * (j + 1), :],
                    a2a_out[j][:].rearrange("(a p) s -> p a s", p=128))
            # 8 parallel accumulators (uses every PSUM bank), d-outer loop
            acc_tiles = (
                [psA.tile([128, 512], F32, tag="acc", name=f"oacc{i}")
                 for i in range(4)]
                + [psS.tile([128, 1024], F32, tag="sc", name="oaccS")]
                + [psX.tile([128, 512], F32, tag="rope", name=f"oaccX{i}")
                   for i in range(2)]
            )
            # combo index -> (psum AP, sc_i, n_i)
            combos = []
            big = acc_tiles[4]
            aps = [acc_tiles[0][:], acc_tiles[1][:], acc_tiles[2][:],
                   acc_tiles[3][:], big[:, 0:512], big[:, 512:1024],
                   acc_tiles[5][:], acc_tiles[6][:]]
            for idx in range(8):
                combos.append((aps[idx], idx // 4, idx % 4))
            for i in range(NKCH):
                wo_t = wop.tile([128, DIM], F32R, tag="wo")
                nc.scalar.dma_start(wo_t[:], wo_e[128 * i:128 * (i + 1), :])
                for (ap, sc_i, n_i) in combos:
                    nc.tensor.matmul(
                        ap, ctx_all[:, i, 128 * sc_i:128 * (sc_i + 1)],
                        wo_t[:, 512 * n_i:512 * (n_i + 1)],
                        start=(i == 0), stop=(i == NKCH - 1))
            for (ap, sc_i, n_i) in combos:
                o_sb = workp.tile([128, 512], F32, tag="osb")
                nc.vector.tensor_copy(o_sb[:], ap)
                nc.sync.dma_start(
                    out_e[128 * sc_i:128 * (sc_i + 1),
                          512 * n_i:512 * (n_i + 1)], o_sb[:])

    nc.compile()
    return nc


def _host_tables():
    pos = np.arange(S, dtype=np.float64)[:, None]
    div = np.exp(np.arange(0, HD, 2, dtype=np.float64)
                 * (-math.log(ROPE_BASE) / HD))
    ang = pos * div                      # [S, 32]
    cos32 = np.cos(ang).T.astype(np.float32)   # [32, S]
    sin32 = np.sin(ang).T.astype(np.float32)
    cosx = np.tile(cos32, (4, 1))        # [128, S]
    sinx = np.tile(sin32, (4, 1))

    # selA: for head slot hh (0/1): out[m] += T1[64*hh + m]
    selA = np.zeros((128, 128), np.float32)
    selB = np.zeros((128, 128), np.float32)
    for hh in range(2):
        for m in range(64):
            selA[64 * hh + m, 64 * hh + m] = 1.0
        for m in range(32):
            selB[64 * hh + m + 32, 64 * hh + m] = -1.0   # new_e -= sin*o
            selB[64 * hh + m, 64 * hh + m + 32] = 1.0    # new_o += sin*e
    ident = np.eye(128, dtype=np.float32)
    onesr = np.zeros((128, 64), np.float32)
    onesr[64, :] = 1.0
    return cosx, sinx, selA, selB, ident, onesr


def _perm_head_cols():
    """Within one 64-col head block: [evens, odds]."""
    p = np.empty(HD, np.int64)
    p[:32] = np.arange(0, HD, 2)
    p[32:] = np.arange(1, HD, 2)
    return p


def _wo_perm_rows():
    perm = np.empty(DIM, np.int64)
    for i in range(NKCH):
        j, u = i // 4, i % 4
        for p in range(128):
            r = 2 * u + p // 64
            dd = p % 64
            perm[128 * i + p] = 64 * (4 * r + j) + dd
    return perm


_CACHE = {}


def kernel(x, mask, wq, wk, wv, wo):
    x = np.asarray(x, dtype=np.float32)
    mask = np.asarray(mask)
    wq = np.asarray(wq, dtype=np.float32)
    wk = np.asarray(wk, dtype=np.float32)
    wv = np.asarray(wv, dtype=np.float32)
    wo = np.asarray(wo, dtype=np.float32)

    state, patterns = _analyze_mask(mask)
    sig = (tuple(tuple(r) for r in state),
           tuple(p.tobytes() for p in patterns))
    if sig not in _CACHE:
        _CACHE[sig] = _build(state, len(patterns))
    nc = _CACHE[sig]

    cosx, sinx, selA, selB, ident, onesr = _host_tables()
    hperm = _perm_head_cols()
    npat = max(len(patterns), 1)
    mb = np.zeros((128, npat * 512), np.float32)
    for pi, pat in enumerate(patterns):
        mb[:, 512 * pi:512 * (pi + 1)] = pat

    xT = np.ascontiguousarray(x.reshape(S, DIM).T)
    wo_p = np.ascontiguousarray(wo[_wo_perm_rows(), :])

    in_maps = []
    for c in range(N_CORES):
        wq_c = np.empty((DIM, NREP * HD), np.float32)
        for hl in range(NREP):
            h = NREP * c + hl
            cols = HD * h + hperm
            wq_c[:, HD * hl:HD * (hl + 1)] = wq[:, cols] * SCALE
        wkv_c = np.empty((DIM, 2 * HD), np.float32)
        wkv_c[:, :HD] = wk[:, HD * c + hperm]
        wkv_c[:, HD:] = wv[:, HD * c:HD * (c + 1)]
        in_maps.append({
            "xT": xT, "wq": np.ascontiguousarray(wq_c),
            "wkv": np.ascontiguousarray(wkv_c), "wo": wo_p,
            "cosx": cosx, "sinx": sinx, "selA": selA, "selB": selB,
            "ident": ident, "onesr": onesr, "maskbias": mb,
        })

    global _LAST_IN_MAPS
    _LAST_IN_MAPS = in_maps
    res = run_bass_kernel_spmd(nc, in_maps, list(range(N_CORES)))
    out = np.concatenate([res.results[c]["out"] for c in range(N_CORES)],
                         axis=0)
    return out.reshape(B, S, DIM).astype(np.float32, copy=False)


# revision 8
# speedup vs baseline: 1.3178x; 1.3178x over previous
"""Trainium2 Bass kernel for nn_LlamaAttention_16372415332980.

Llama GQA attention (B=1, S=2048, D=2048, NH=32, NKV=8, HD=64) with RoPE and
causal mask, tensor-parallel over 8 NeuronCores: core c owns kv-head c
(= query heads 4c..4c+3). Per-core pipeline, all matmuls fp32r:

  1. projections: qT/kT/vT = (w.T @ xT-chunks), x transposed host-side
  2. RoPE: 2 DVE muls (cos/sin) + one PE "select+swap" combine matmul per head
     (weight columns host-permuted so even/odd lanes form 32-row blocks)
  3. scoresT[kk,sq] = kT.T @ qT per 128x512 block, causal blocks skipped,
     mask applied by accumulating a -30000 bias tile via identity matmul
  4. exp on ScalarE (PSUM->SBUF); A@V with a ones-row appended to V so the
     softmax denominator comes out of the same matmul (row 64 of ctx PSUM)
  5. normalize, AllToAll the per-head context (seq-sharding it), and
  6. output projection d-outer over 8 PSUM accumulators with row-permuted wo;
     each core emits output rows [256c, 256c+256); host concatenates.
"""

import math
import sys

for _p in ("/opt/trn_rl_repo",):
    if _p not in sys.path:
        sys.path.insert(0, _p)

import numpy as np
import ml_dtypes
BF = ml_dtypes.bfloat16

import concourse.bacc as bacc
import concourse.bass as bass
import concourse.mybir as mybir
import concourse.tile as tile
from concourse.bass_utils import run_bass_kernel_spmd

B, S, DIM = 1, 2048, 2048
NH, NKV, HD = 32, 8, 64
NREP = NH // NKV
N_CORES = 8
SCALE = 1.0 / math.sqrt(HD)
ROPE_BASE = 10000.0

F32 = mybir.dt.float32
F32R = mybir.dt.float32r
BF16 = mybir.dt.bfloat16

NKK = S // 128      # 16 key tiles of 128
NSQ = S // 512      # 4 query chunks of 512
NKCH = DIM // 128   # 16 contraction chunks
SKIP, FULL = -1, -2
NEG = -30000.0


def _analyze_mask(mask):
    """Classify each [128 kk, 512 sq] block of the (head-shared) mask.

    Returns (state[t][c], patterns): state is SKIP / FULL / pattern index;
    patterns[p] is a [128, 512] float32 0/-30000 bias tile.
    scoresT[kk, sq] is masked where mask[0,0,sq,kk] == 0.
    """
    m = np.asarray(mask).reshape(S, S)
    patterns = []
    index = {}
    state = [[SKIP] * NSQ for _ in range(NKK)]
    for t in range(NKK):
        for c in range(NSQ):
            blk = m[c * 512:(c + 1) * 512, t * 128:(t + 1) * 128].T  # [128,512]
            if not blk.any():
                state[t][c] = SKIP
            elif blk.all():
                state[t][c] = FULL
            else:
                key = blk.tobytes()
                if key not in index:
                    index[key] = len(patterns)
                    patterns.append((1.0 - blk.astype(np.float32)) * NEG)
                state[t][c] = index[key]
    return state, patterns


def _build(state, npat):
    """Build + trace the SPMD Tile program for a given mask block structure."""
    nc = bacc.Bacc("TRN2", target_bir_lowering=False, debug=False,
                   num_devices=N_CORES)

    # ---- external inputs (per-core, host-prepped) ----
    xT_e = nc.dram_tensor("xT", [DIM, S], BF16, kind="ExternalInput")
    wq_e = nc.dram_tensor("wq", [DIM, NREP * HD], BF16, kind="ExternalInput")
    wkv_e = nc.dram_tensor("wkv", [DIM, 2 * HD], BF16, kind="ExternalInput")
    wo_e = nc.dram_tensor("wo", [DIM, DIM], BF16, kind="ExternalInput")
    cos_e = nc.dram_tensor("cosx", [128, S], F32, kind="ExternalInput")
    sin_e = nc.dram_tensor("sinx", [128, S], F32, kind="ExternalInput")
    selA_e = nc.dram_tensor("selA", [128, 128], BF16, kind="ExternalInput")
    selB_e = nc.dram_tensor("selB", [128, 128], BF16, kind="ExternalInput")
    ident_e = nc.dram_tensor("ident", [128, 128], BF16, kind="ExternalInput")
    onesr_e = nc.dram_tensor("onesr", [128, 64], F32R, kind="ExternalInput")
    mb_e = nc.dram_tensor("maskbias", [128, max(npat, 1) * 512], BF16,
                          kind="ExternalInput")
    out_e = nc.dram_tensor("out", [S // N_CORES, DIM], F32, kind="ExternalOutput")

    rg = [list(range(N_CORES))]

    with tile.TileContext(nc) as tc:
        with (
            tc.tile_pool(name="const", bufs=1) as constp,
            tc.tile_pool(name="wqp", bufs=1) as wqp,
            tc.tile_pool(name="xsp", bufs=2) as xsp,
            tc.tile_pool(name="qkp", bufs=1) as qkp,
            tc.tile_pool(name="ropep", bufs=2) as ropep,
            tc.tile_pool(name="expp", bufs=2) as expp,
            tc.tile_pool(name="ctxp", bufs=2) as ctxp,
            tc.tile_pool(name="workp", bufs=2) as workp,
            tc.tile_pool(name="wop", bufs=3) as wop,
            tc.tile_pool(name="ctxall", bufs=1) as ctxallp,
            tc.tile_pool(name="psA", bufs=4, space="PSUM") as psA,   # 4 banks
            tc.tile_pool(name="psS", bufs=1, space="PSUM") as psS,   # 2 banks
            tc.tile_pool(name="psX", bufs=2, space="PSUM") as psX,   # 2 banks
            tc.tile_pool(name="dram", bufs=1, space="DRAM") as dram,
        ):
            # ---- constants / weights to SBUF ----
            cos_s = constp.tile([128, S], F32, tag="cos")
            sin_s = constp.tile([128, S], F32, tag="sin")
            selA = constp.tile([128, 128], BF16, tag="selA")
            selB = constp.tile([128, 128], BF16, tag="selB")
            ident = constp.tile([128, 128], BF16, tag="ident")
            onesr = constp.tile([128, 64], F32R, tag="onesr")
            mb_s = constp.tile([128, max(npat, 1) * 512], BF16, tag="mb")
            nc.sync.dma_start(cos_s[:], cos_e[:])
            nc.sync.dma_start(sin_s[:], sin_e[:])
            nc.sync.dma_start(selA[:], selA_e[:])
            nc.sync.dma_start(selB[:], selB_e[:])
            nc.sync.dma_start(ident[:], ident_e[:])
            nc.sync.dma_start(onesr[:], onesr_e[:])
            nc.sync.dma_start(mb_s[:], mb_e[:])

            wq_s = wqp.tile([128, NKCH, 256], BF16, tag="wq")
            wkv_s = wqp.tile([128, NKCH, 128], BF16, tag="wkv")
            nc.sync.dma_start(
                wq_s[:], wq_e[:].rearrange("(a p) n -> p a n", p=128))
            nc.sync.dma_start(
                wkv_s[:], wkv_e[:].rearrange("(a p) n -> p a n", p=128))

            # ---- persistent activation tiles ----
            qT = [qkp.tile([64, S], BF16, tag=f"qT{h}", name=f"qT{h}")
                  for h in range(NREP)]
            kT = qkp.tile([64, S], BF16, tag="kT")
            v_aug = qkp.tile([128, NKK, 65], BF16, tag="vaug")
            nc.vector.memset(v_aug[:, :, 64], 1.0)

            # ================= phase 1: projections + rope =================
            # groups: 0 -> q heads 0,1 ; 1 -> q heads 2,3 ; 2 -> [k | v]
            for c in range(NSQ):
                xs = []
                for kq in range(4):  # contraction quarters (4 k-chunks each)
                    x_t = xsp.tile([128, 4, 512], BF16, tag="xs")
                    nc.sync.dma_start(
                        x_t[:],
                        xT_e[512 * kq:512 * (kq + 1),
                             512 * c:512 * (c + 1)].rearrange(
                                 "(a p) s -> p a s", p=128))
                    xs.append(x_t)
                for g in range(3):
                    acc = psA.tile([128, 512], F32, tag="acc")
                    for kq in range(4):
                        for a in range(4):
                            k = 4 * kq + a
                            w_ap = (wq_s[:, k, 128 * g:128 * (g + 1)]
                                    if g < 2 else wkv_s[:, k, :])
                            nc.tensor.matmul(
                                acc[:], w_ap, xs[kq][:, a, :],
                                start=(k == 0), stop=(k == NKCH - 1))
                    # rope: T1 = acc*cos, T2 = acc*sin (both [128,512])
                    t1 = ropep.tile([128, 512], BF16, tag="t1")
                    t2 = ropep.tile([128, 512], BF16, tag="t2")
                    cs = slice(512 * c, 512 * (c + 1))
                    if g < 2:
                        nc.vector.tensor_tensor(
                            t1[:], acc[:], cos_s[:, cs], mybir.AluOpType.mult)
                        nc.vector.tensor_tensor(
                            t2[:], acc[:], sin_s[:, cs], mybir.AluOpType.mult)
                        for hh in range(2):
                            h = 2 * g + hh
                            rp = psX.tile([64, 512], F32, tag="rope")
                            nc.tensor.matmul(
                                rp[:], selA[:, 64 * hh:64 * hh + 64], t1[:],
                                start=True, stop=False)
                            nc.tensor.matmul(
                                rp[:], selB[:, 64 * hh:64 * hh + 64], t2[:],
                                start=False, stop=True)
                            nc.scalar.copy(qT[h][:, cs], rp[:])
                    else:
                        nc.vector.tensor_tensor(
                            t1[0:64, :], acc[0:64, :], cos_s[0:64, cs],
                            mybir.AluOpType.mult)
                        nc.vector.tensor_tensor(
                            t2[0:64, :], acc[0:64, :], sin_s[0:64, cs],
                            mybir.AluOpType.mult)
                        rp = psX.tile([64, 512], F32, tag="rope")
                        nc.tensor.matmul(rp[:], selA[0:64, 0:64], t1[0:64, :],
                                         start=True, stop=False)
                        nc.tensor.matmul(rp[:], selB[0:64, 0:64], t2[0:64, :],
                                         start=False, stop=True)
                        nc.scalar.copy(kT[:, cs], rp[:])
                        # v: rows 64:127 of acc -> transpose into v_aug
                        vt = workp.tile([128, 512], BF16, tag="vtmp")
                        nc.vector.tensor_copy(vt[64:128, :], acc[64:128, :])
                        for u in range(4):
                            tp = psX.tile([128, 64], BF16, tag="rope")
                            nc.tensor.transpose(
                                tp[:], vt[64:128, 128 * u:128 * (u + 1)],
                                ident[64:128, 64:128])
                            nc.vector.tensor_copy(
                                v_aug[:, 4 * c + u, 0:64], tp[:])

            # ================= phase 2+3: attention per head ===============
            first_t = [min(t for t in range(NKK) if state[t][c] != SKIP)
                       for c in range(NSQ)]
            last_t = [max(t for t in range(NKK) if state[t][c] != SKIP)
                      for c in range(NSQ)]
            a2a_out = []
            for h in range(NREP):
                ctx_ps = [psA.tile([65, 512], F32, tag="acc", name=f"ctx{h}_{c}")
                          for c in range(NSQ)]
                for t in range(NKK):
                    for p in range(2):
                        cs_l = [c for c in (2 * p, 2 * p + 1)
                                if state[t][c] != SKIP]
                        if not cs_l:
                            continue
                        sc = psS.tile([128, 1024], F32, tag="sc")
                        for c in cs_l:
                            o = (c - 2 * p) * 512
                            st = state[t][c]
                            nc.tensor.matmul(
                                sc[:, o:o + 512],
                                kT[:, 128 * t:128 * (t + 1)],
                                qT[h][:, 512 * c:512 * (c + 1)],
                                start=True, stop=(st == FULL))
                            if st != FULL:
                                nc.tensor.matmul(
                                    sc[:, o:o + 512], ident[:],
                                    mb_s[:, 512 * st:512 * (st + 1)],
                                    start=False, stop=True)
                        lo = (cs_l[0] - 2 * p) * 512
                        hi = (cs_l[-1] - 2 * p) * 512 + 512
                        ex = expp.tile([128, 1024], BF16, tag="ex")
                        nc.scalar.activation(
                            ex[:, lo:hi], sc[:, lo:hi],
                            mybir.ActivationFunctionType.Exp)
                        for c in cs_l:
                            o = (c - 2 * p) * 512
                            nc.tensor.matmul(
                                ctx_ps[c][:], v_aug[:, t, :],
                                ex[:, o:o + 512],
                                start=(t == first_t[c]), stop=(t == last_t[c]))
                # normalize + stage for AllToAll
                ctx_sb = ctxp.tile([64, S], BF16, tag="ctxsb")
                for c in range(NSQ):
                    rec = workp.tile([65, 512], F32R, tag="rec")
                    with nc.allow_low_precision(reason="f32r recip feeds matmul"):
                        nc.vector.reciprocal(rec[64:65, :], ctx_ps[c][64:65, :])
                    bc = psX.tile([64, 512], F32, tag="rope")
                    nc.tensor.matmul(bc[:], onesr[64:65, :], rec[64:65, :],
                                     start=True, stop=True)
                    bc_sb = workp.tile([64, 512], F32, tag="bcsb")
                    nc.vector.tensor_copy(bc_sb[:], bc[:])
                    nc.vector.tensor_tensor(
                        ctx_sb[:, 512 * c:512 * (c + 1)],
                        ctx_ps[c][0:64, :], bc_sb[:], mybir.AluOpType.mult)
                a_in = dram.tile([8 * 64, 256], BF16, tag=f"a2ai{h}")
                a_out = dram.tile([8 * 64, 256], BF16, tag=f"a2ao{h}")
                a2a_out.append(a_out)
                nc.sync.dma_start(
                    a_in[:].rearrange("(j d) s -> d j s", j=8),
                    ctx_sb[:].rearrange("d (j s) -> d j s", j=8))
                nc.gpsimd.collective_compute(
                    "AllToAll", mybir.AluOpType.bypass, replica_groups=rg,
                    ins=[a_in.opt()], outs=[a_out.opt()])

            # ================= phase 4: output projection ==================
            ctx_all = ctxallp.tile([128, NKCH, 256], BF16, tag="call")
            for j in range(NREP):
                nc.sync.dma_start(
                    ctx_all[:, 4 * j:4 * (j + 1), :],
                    a2a_out[j][:].rearrange("(a p) s -> p a s", p=128))
            # 8 parallel accumulators (uses every PSUM bank), d-outer loop
            acc_tiles = (
                [psA.tile([128, 512], F32, tag="acc", name=f"oacc{i}")
                 for i in range(4)]
                + [psS.tile([128, 1024], F32, tag="sc", name="oaccS")]
                + [psX.tile([128, 512], F32, tag="rope", name=f"oaccX{i}")
                   for i in range(2)]
            )
            # combo index -> (psum AP, sc_i, n_i)
            combos = []
            big = acc_tiles[4]
            aps = [acc_tiles[0][:], acc_tiles[1][:], acc_tiles[2][:],
                   acc_tiles[3][:], big[:, 0:512], big[:, 512:1024],
                   acc_tiles[5][:], acc_tiles[6][:]]
            for idx in range(8):
                combos.append((aps[idx], idx // 4, idx % 4))
            for i in range(NKCH):
                wo_t = wop.tile([128, DIM], BF16, tag="wo")
                nc.scalar.dma_start(wo_t[:], wo_e[128 * i:128 * (i + 1), :])
                for (ap, sc_i, n_i) in combos:
                    nc.tensor.matmul(
                        ap, ctx_all[:, i, 128 * sc_i:128 * (sc_i + 1)],
                        wo_t[:, 512 * n_i:512 * (n_i + 1)],
                        start=(i == 0), stop=(i == NKCH - 1))
            for (ap, sc_i, n_i) in combos:
                o_sb = workp.tile([128, 512], F32, tag="osb")
                nc.vector.tensor_copy(o_sb[:], ap)
                nc.sync.dma_start(
                    out_e[128 * sc_i:128 * (sc_i + 1),
                          512 * n_i:512 * (n_i + 1)], o_sb[:])

    nc.compile()
    return nc


def _host_tables():
    pos = np.arange(S, dtype=np.float64)[:, None]
    div = np.exp(np.arange(0, HD, 2, dtype=np.float64)
                 * (-math.log(ROPE_BASE) / HD))
    ang = pos * div                      # [S, 32]
    cos32 = np.cos(ang).T.astype(np.float32)   # [32, S]
    sin32 = np.sin(ang).T.astype(np.float32)
    cosx = np.tile(cos32, (4, 1))        # [128, S]
    sinx = np.tile(sin32, (4, 1))

    # selA: for head slot hh (0/1): out[m] += T1[64*hh + m]
    selA = np.zeros((128, 128), np.float32)
    selB = np.zeros((128, 128), np.float32)
    for hh in range(2):
        for m in range(64):
            selA[64 * hh + m, 64 * hh + m] = 1.0
        for m in range(32):
            selB[64 * hh + m + 32, 64 * hh + m] = -1.0   # new_e -= sin*o
            selB[64 * hh + m, 64 * hh + m + 32] = 1.0    # new_o += sin*e
    ident = np.eye(128, dtype=np.float32)
    onesr = np.zeros((128, 64), np.float32)
    onesr[64, :] = 1.0
    return cosx, sinx, selA, selB, ident, onesr


def _perm_head_cols():
    """Within one 64-col head block: [evens, odds]."""
    p = np.empty(HD, np.int64)
    p[:32] = np.arange(0, HD, 2)
    p[32:] = np.arange(1, HD, 2)
    return p


def _wo_perm_rows():
    perm = np.empty(DIM, np.int64)
    for i in range(NKCH):
        j, u = i // 4, i % 4
        for p in range(128):
            r = 2 * u + p // 64
            dd = p % 64
            perm[128 * i + p] = 64 * (4 * r + j) + dd
    return perm


_CACHE = {}


def kernel(x, mask, wq, wk, wv, wo):
    x = np.asarray(x, dtype=np.float32)
    mask = np.asarray(mask)
    wq = np.asarray(wq, dtype=np.float32)
    wk = np.asarray(wk, dtype=np.float32)
    wv = np.asarray(wv, dtype=np.float32)
    wo = np.asarray(wo, dtype=np.float32)

    state, patterns = _analyze_mask(mask)
    sig = (tuple(tuple(r) for r in state),
           tuple(p.tobytes() for p in patterns))
    if sig not in _CACHE:
        _CACHE[sig] = _build(state, len(patterns))
    nc = _CACHE[sig]

    cosx, sinx, selA, selB, ident, onesr = _host_tables()
    hperm = _perm_head_cols()
    npat = max(len(patterns), 1)
    mb = np.zeros((128, npat * 512), np.float32)
    for pi, pat in enumerate(patterns):
        mb[:, 512 * pi:512 * (pi + 1)] = pat

    xT_b = np.ascontiguousarray(x.reshape(S, DIM).T).astype(BF)
    wo_b = np.ascontiguousarray(wo[_wo_perm_rows(), :]).astype(BF)

    in_maps = []
    for c in range(N_CORES):
        wq_c = np.empty((DIM, NREP * HD), np.float32)
        for hl in range(NREP):
            h = NREP * c + hl
            cols = HD * h + hperm
            wq_c[:, HD * hl:HD * (hl + 1)] = wq[:, cols] * SCALE
        wkv_c = np.empty((DIM, 2 * HD), np.float32)
        wkv_c[:, :HD] = wk[:, HD * c + hperm]
        wkv_c[:, HD:] = wv[:, HD * c:HD * (c + 1)]
        in_maps.append({
            "xT": xT_b, "wq": np.ascontiguousarray(wq_c).astype(BF),
            "wkv": np.ascontiguousarray(wkv_c).astype(BF), "wo": wo_b,
            "cosx": cosx, "sinx": sinx, "selA": selA.astype(BF),
            "selB": selB.astype(BF), "ident": ident.astype(BF),
            "onesr": onesr, "maskbias": mb.astype(BF),
        })

    global _LAST_IN_MAPS
    _LAST_IN_MAPS = in_maps
    res = run_bass_kernel_spmd(nc, in_maps, list(range(N_CORES)))
    out = np.concatenate([res.results[c]["out"] for c in range(N_CORES)],
                         axis=0)
    return out.reshape(B, S, DIM).astype(np.float32, copy=False)


# revision 9
# speedup vs baseline: 1.6711x; 1.2681x over previous
"""Trainium2 Bass kernel for nn_LlamaAttention_16372415332980.

Llama GQA attention (B=1, S=2048, D=2048, NH=32, NKV=8, HD=64) with RoPE and
causal mask, tensor-parallel over 8 NeuronCores: core c owns kv-head c
(= query heads 4c..4c+3). Per-core pipeline, all matmuls fp32r:

  1. projections: qT/kT/vT = (w.T @ xT-chunks), x transposed host-side
  2. RoPE: 2 DVE muls (cos/sin) + one PE "select+swap" combine matmul per head
     (weight columns host-permuted so even/odd lanes form 32-row blocks)
  3. scoresT[kk,sq] = kT.T @ qT per 128x512 block, causal blocks skipped,
     mask applied by accumulating a -30000 bias tile via identity matmul
  4. exp on ScalarE (PSUM->SBUF); A@V with a ones-row appended to V so the
     softmax denominator comes out of the same matmul (row 64 of ctx PSUM)
  5. normalize, AllToAll the per-head context (seq-sharding it), and
  6. output projection d-outer over 8 PSUM accumulators with row-permuted wo;
     each core emits output rows [256c, 256c+256); host concatenates.
"""

import math
import sys

for _p in ("/opt/trn_rl_repo",):
    if _p not in sys.path:
        sys.path.insert(0, _p)

import numpy as np
import ml_dtypes
BF = ml_dtypes.bfloat16

import concourse.bacc as bacc
import concourse.bass as bass
import concourse.mybir as mybir
import concourse.tile as tile
from concourse.bass_utils import run_bass_kernel_spmd

B, S, DIM = 1, 2048, 2048
NH, NKV, HD = 32, 8, 64
NREP = NH // NKV
N_CORES = 8
SCALE = 1.0 / math.sqrt(HD)
ROPE_BASE = 10000.0

F32 = mybir.dt.float32
F32R = mybir.dt.float32r
BF16 = mybir.dt.bfloat16

NKK = S // 128      # 16 key tiles of 128
NSQ = S // 512      # 4 query chunks of 512
NKCH = DIM // 128   # 16 contraction chunks
SKIP, FULL = -1, -2
NEG = -30000.0


def _analyze_mask(mask):
    """Classify each [128 kk, 512 sq] block of the (head-shared) mask.

    Returns (state[t][c], patterns): state is SKIP / FULL / pattern index;
    patterns[p] is a [128, 512] float32 0/-30000 bias tile.
    scoresT[kk, sq] is masked where mask[0,0,sq,kk] == 0.
    """
    m = np.asarray(mask).reshape(S, S)
    patterns = []
    index = {}
    state = [[SKIP] * NSQ for _ in range(NKK)]
    for t in range(NKK):
        for c in range(NSQ):
            blk = m[c * 512:(c + 1) * 512, t * 128:(t + 1) * 128].T  # [128,512]
            if not blk.any():
                state[t][c] = SKIP
            elif blk.all():
                state[t][c] = FULL
            else:
                key = blk.tobytes()
                if key not in index:
                    index[key] = len(patterns)
                    patterns.append((1.0 - blk.astype(np.float32)) * NEG)
                state[t][c] = index[key]
    return state, patterns


def _build(state, npat):
    """Build + trace the SPMD Tile program for a given mask block structure."""
    nc = bacc.Bacc("TRN2", target_bir_lowering=False, debug=False,
                   num_devices=N_CORES)

    # ---- external inputs (per-core, host-prepped) ----
    xT_e = nc.dram_tensor("xT", [DIM, S], BF16, kind="ExternalInput")
    wq_e = nc.dram_tensor("wq", [DIM, NREP * HD], BF16, kind="ExternalInput")
    wkv_e = nc.dram_tensor("wkv", [DIM, 2 * HD], BF16, kind="ExternalInput")
    wo_e = nc.dram_tensor("wo", [DIM, DIM], BF16, kind="ExternalInput")
    cos_e = nc.dram_tensor("cosx", [128, S], F32, kind="ExternalInput")
    sin_e = nc.dram_tensor("sinx", [128, S], F32, kind="ExternalInput")
    selA_e = nc.dram_tensor("selA", [128, 128], BF16, kind="ExternalInput")
    selB_e = nc.dram_tensor("selB", [128, 128], BF16, kind="ExternalInput")
    ident_e = nc.dram_tensor("ident", [128, 128], BF16, kind="ExternalInput")
    onesr_e = nc.dram_tensor("onesr", [128, 64], F32R, kind="ExternalInput")
    mb_e = nc.dram_tensor("maskbias", [128, max(npat, 1) * 512], BF16,
                          kind="ExternalInput")
    out_e = nc.dram_tensor("out", [S // N_CORES, DIM], F32, kind="ExternalOutput")

    rg = [list(range(N_CORES))]

    with tile.TileContext(nc) as tc:
        with (
            tc.tile_pool(name="const", bufs=1) as constp,
            tc.tile_pool(name="wqp", bufs=1) as wqp,
            tc.tile_pool(name="xsp", bufs=2) as xsp,
            tc.tile_pool(name="qkp", bufs=1) as qkp,
            tc.tile_pool(name="ropep", bufs=2) as ropep,
            tc.tile_pool(name="expp", bufs=3) as expp,
            tc.tile_pool(name="ctxp", bufs=2) as ctxp,
            tc.tile_pool(name="workp", bufs=2) as workp,
            tc.tile_pool(name="wop", bufs=3) as wop,
            tc.tile_pool(name="ctxall", bufs=1) as ctxallp,
            tc.tile_pool(name="psA", bufs=4, space="PSUM") as psA,   # 4 banks
            tc.tile_pool(name="psS", bufs=3, space="PSUM") as psS,   # 3 banks
            tc.tile_pool(name="psX", bufs=1, space="PSUM") as psX,   # 1 bank
            tc.tile_pool(name="dram", bufs=1, space="DRAM") as dram,
        ):
            # ---- constants / weights to SBUF ----
            cos_s = constp.tile([128, S], F32, tag="cos")
            sin_s = constp.tile([128, S], F32, tag="sin")
            selA = constp.tile([128, 128], BF16, tag="selA")
            selB = constp.tile([128, 128], BF16, tag="selB")
            ident = constp.tile([128, 128], BF16, tag="ident")
            onesr = constp.tile([128, 64], F32R, tag="onesr")
            mb_s = constp.tile([128, max(npat, 1) * 512], BF16, tag="mb")
            nc.sync.dma_start(cos_s[:], cos_e[:])
            nc.sync.dma_start(sin_s[:], sin_e[:])
            nc.sync.dma_start(selA[:], selA_e[:])
            nc.sync.dma_start(selB[:], selB_e[:])
            nc.sync.dma_start(ident[:], ident_e[:])
            nc.sync.dma_start(onesr[:], onesr_e[:])
            nc.sync.dma_start(mb_s[:], mb_e[:])

            wq_s = wqp.tile([128, NKCH, 256], BF16, tag="wq")
            wkv_s = wqp.tile([128, NKCH, 128], BF16, tag="wkv")
            nc.sync.dma_start(
                wq_s[:], wq_e[:].rearrange("(a p) n -> p a n", p=128))
            nc.sync.dma_start(
                wkv_s[:], wkv_e[:].rearrange("(a p) n -> p a n", p=128))

            # ---- persistent activation tiles ----
            qT = [qkp.tile([64, S], BF16, tag=f"qT{h}", name=f"qT{h}")
                  for h in range(NREP)]
            kT = qkp.tile([64, S], BF16, tag="kT")
            v_aug = qkp.tile([128, NKK, 65], BF16, tag="vaug")
            nc.vector.memset(v_aug[:, :, 64], 1.0)

            # ================= phase 1: projections + rope =================
            # groups: 0 -> q heads 0,1 ; 1 -> q heads 2,3 ; 2 -> [k | v]
            for c in range(NSQ):
                xs = []
                for kq in range(4):  # contraction quarters (4 k-chunks each)
                    x_t = xsp.tile([128, 4, 512], BF16, tag="xs")
                    nc.sync.dma_start(
                        x_t[:],
                        xT_e[512 * kq:512 * (kq + 1),
                             512 * c:512 * (c + 1)].rearrange(
                                 "(a p) s -> p a s", p=128))
                    xs.append(x_t)
                for g in range(3):
                    acc = psA.tile([128, 512], F32, tag="acc")
                    for kq in range(4):
                        for a in range(4):
                            k = 4 * kq + a
                            w_ap = (wq_s[:, k, 128 * g:128 * (g + 1)]
                                    if g < 2 else wkv_s[:, k, :])
                            nc.tensor.matmul(
                                acc[:], w_ap, xs[kq][:, a, :],
                                start=(k == 0), stop=(k == NKCH - 1))
                    # rope: T1 = acc*cos, T2 = acc*sin (both [128,512])
                    t1 = ropep.tile([128, 512], BF16, tag="t1")
                    t2 = ropep.tile([128, 512], BF16, tag="t2")
                    cs = slice(512 * c, 512 * (c + 1))
                    if g < 2:
                        nc.vector.tensor_tensor(
                            t1[:], acc[:], cos_s[:, cs], mybir.AluOpType.mult)
                        nc.vector.tensor_tensor(
                            t2[:], acc[:], sin_s[:, cs], mybir.AluOpType.mult)
                        for hh in range(2):
                            h = 2 * g + hh
                            rp = psX.tile([64, 512], F32, tag="rope")
                            nc.tensor.matmul(
                                rp[:], selA[:, 64 * hh:64 * hh + 64], t1[:],
                                start=True, stop=False)
                            nc.tensor.matmul(
                                rp[:], selB[:, 64 * hh:64 * hh + 64], t2[:],
                                start=False, stop=True)
                            nc.scalar.copy(qT[h][:, cs], rp[:])
                    else:
                        nc.vector.tensor_tensor(
                            t1[0:64, :], acc[0:64, :], cos_s[0:64, cs],
                            mybir.AluOpType.mult)
                        nc.vector.tensor_tensor(
                            t2[0:64, :], acc[0:64, :], sin_s[0:64, cs],
                            mybir.AluOpType.mult)
                        rp = psX.tile([64, 512], F32, tag="rope")
                        nc.tensor.matmul(rp[:], selA[0:64, 0:64], t1[0:64, :],
                                         start=True, stop=False)
                        nc.tensor.matmul(rp[:], selB[0:64, 0:64], t2[0:64, :],
                                         start=False, stop=True)
                        nc.scalar.copy(kT[:, cs], rp[:])
                        # v: rows 64:127 of acc -> transpose into v_aug
                        vt = workp.tile([128, 512], BF16, tag="vtmp")
                        nc.vector.tensor_copy(vt[64:128, :], acc[64:128, :])
                        for u in range(4):
                            tp = psX.tile([128, 64], BF16, tag="rope")
                            nc.tensor.transpose(
                                tp[:], vt[64:128, 128 * u:128 * (u + 1)],
                                ident[64:128, 64:128])
                            nc.vector.tensor_copy(
                                v_aug[:, 4 * c + u, 0:64], tp[:])

            # ================= phase 2+3: attention per head ===============
            first_t = [min(t for t in range(NKK) if state[t][c] != SKIP)
                       for c in range(NSQ)]
            last_t = [max(t for t in range(NKK) if state[t][c] != SKIP)
                      for c in range(NSQ)]
            a2a_out = []
            for h in range(NREP):
                ctx_ps = [psA.tile([65, 512], F32, tag="acc", name=f"ctx{h}_{c}")
                          for c in range(NSQ)]
                for t in range(NKK):
                    for c in range(NSQ):
                        st = state[t][c]
                        if st == SKIP:
                            continue
                        sc = psS.tile([128, 512], F32, tag="sc")
                        nc.tensor.matmul(
                            sc[:], kT[:, 128 * t:128 * (t + 1)],
                            qT[h][:, 512 * c:512 * (c + 1)],
                            start=True, stop=(st == FULL))
                        if st != FULL:
                            nc.tensor.matmul(
                                sc[:], ident[:],
                                mb_s[:, 512 * st:512 * (st + 1)],
                                start=False, stop=True)
                        ex = expp.tile([128, 512], BF16, tag="ex")
                        nc.scalar.activation(
                            ex[:], sc[:],
                            mybir.ActivationFunctionType.Exp)
                        nc.tensor.matmul(
                            ctx_ps[c][:], v_aug[:, t, :], ex[:],
                            start=(t == first_t[c]), stop=(t == last_t[c]))
                # normalize + stage for AllToAll
                ctx_sb = ctxp.tile([64, S], BF16, tag="ctxsb")
                for c in range(NSQ):
                    rec = workp.tile([65, 512], F32R, tag="rec")
                    with nc.allow_low_precision(reason="f32r recip feeds matmul"):
                        nc.vector.reciprocal(rec[64:65, :], ctx_ps[c][64:65, :])
                    bc = psX.tile([64, 512], F32, tag="rope")
                    nc.tensor.matmul(bc[:], onesr[64:65, :], rec[64:65, :],
                                     start=True, stop=True)
                    bc_sb = workp.tile([64, 512], F32, tag="bcsb")
                    nc.vector.tensor_copy(bc_sb[:], bc[:])
                    nc.vector.tensor_tensor(
                        ctx_sb[:, 512 * c:512 * (c + 1)],
                        ctx_ps[c][0:64, :], bc_sb[:], mybir.AluOpType.mult)
                a_in = dram.tile([8 * 64, 256], BF16, tag=f"a2ai{h}")
                a_out = dram.tile([8 * 64, 256], BF16, tag=f"a2ao{h}")
                a2a_out.append(a_out)
                nc.sync.dma_start(
                    a_in[:].rearrange("(j d) s -> d j s", j=8),
                    ctx_sb[:].rearrange("d (j s) -> d j s", j=8))
                nc.gpsimd.collective_compute(
                    "AllToAll", mybir.AluOpType.bypass, replica_groups=rg,
                    ins=[a_in.opt()], outs=[a_out.opt()])

            # ================= phase 4: output projection ==================
            ctx_all = ctxallp.tile([128, NKCH, 256], BF16, tag="call")
            for j in range(NREP):
                nc.sync.dma_start(
                    ctx_all[:, 4 * j:4 * (j + 1), :],
                    a2a_out[j][:].rearrange("(a p) s -> p a s", p=128))
            # 8 parallel accumulators (uses every PSUM bank), d-outer loop
            acc_tiles = (
                [psA.tile([128, 512], F32, tag="acc", name=f"oacc{i}")
                 for i in range(4)]
                + [psS.tile([128, 512], F32, tag="sc", name=f"oaccS{i}")
                   for i in range(3)]
                + [psX.tile([128, 512], F32, tag="rope", name="oaccX0")]
            )
            combos = []
            aps = [t[:] for t in acc_tiles]
            for idx in range(8):
                combos.append((aps[idx], idx // 4, idx % 4))
            for i in range(NKCH):
                wo_t = wop.tile([128, DIM], BF16, tag="wo")
                nc.scalar.dma_start(wo_t[:], wo_e[128 * i:128 * (i + 1), :])
                for (ap, sc_i, n_i) in combos:
                    nc.tensor.matmul(
                        ap, ctx_all[:, i, 128 * sc_i:128 * (sc_i + 1)],
                        wo_t[:, 512 * n_i:512 * (n_i + 1)],
                        start=(i == 0), stop=(i == NKCH - 1))
            for (ap, sc_i, n_i) in combos:
                o_sb = workp.tile([128, 512], F32, tag="osb")
                nc.vector.tensor_copy(o_sb[:], ap)
                nc.sync.dma_start(
                    out_e[128 * sc_i:128 * (sc_i + 1),
                          512 * n_i:512 * (n_i + 1)], o_sb[:])

    nc.compile()
    return nc


def _host_tables():
    pos = np.arange(S, dtype=np.float64)[:, None]
    div = np.exp(np.arange(0, HD, 2, dtype=np.float64)
                 * (-math.log(ROPE_BASE) / HD))
    ang = pos * div                      # [S, 32]
    cos32 = np.cos(ang).T.astype(np.float32)   # [32, S]
    sin32 = np.sin(ang).T.astype(np.float32)
    cosx = np.tile(cos32, (4, 1))        # [128, S]
    sinx = np.tile(sin32, (4, 1))

    # selA: for head slot hh (0/1): out[m] += T1[64*hh + m]
    selA = np.zeros((128, 128), np.float32)
    selB = np.zeros((128, 128), np.float32)
    for hh in range(2):
        for m in range(64):
            selA[64 * hh + m, 64 * hh + m] = 1.0
        for m in range(32):
            selB[64 * hh + m + 32, 64 * hh + m] = -1.0   # new_e -= sin*o
            selB[64 * hh + m, 64 * hh + m + 32] = 1.0    # new_o += sin*e
    ident = np.eye(128, dtype=np.float32)
    onesr = np.zeros((128, 64), np.float32)
    onesr[64, :] = 1.0
    return cosx, sinx, selA, selB, ident, onesr


def _perm_head_cols():
    """Within one 64-col head block: [evens, odds]."""
    p = np.empty(HD, np.int64)
    p[:32] = np.arange(0, HD, 2)
    p[32:] = np.arange(1, HD, 2)
    return p


def _wo_perm_rows():
    perm = np.empty(DIM, np.int64)
    for i in range(NKCH):
        j, u = i // 4, i % 4
        for p in range(128):
            r = 2 * u + p // 64
            dd = p % 64
            perm[128 * i + p] = 64 * (4 * r + j) + dd
    return perm


_CACHE = {}


def kernel(x, mask, wq, wk, wv, wo):
    x = np.asarray(x, dtype=np.float32)
    mask = np.asarray(mask)
    wq = np.asarray(wq, dtype=np.float32)
    wk = np.asarray(wk, dtype=np.float32)
    wv = np.asarray(wv, dtype=np.float32)
    wo = np.asarray(wo, dtype=np.float32)

    state, patterns = _analyze_mask(mask)
    sig = (tuple(tuple(r) for r in state),
           tuple(p.tobytes() for p in patterns))
    if sig not in _CACHE:
        _CACHE[sig] = _build(state, len(patterns))
    nc = _CACHE[sig]

    cosx, sinx, selA, selB, ident, onesr = _host_tables()
    hperm = _perm_head_cols()
    npat = max(len(patterns), 1)
    mb = np.zeros((128, npat * 512), np.float32)
    for pi, pat in enumerate(patterns):
        mb[:, 512 * pi:512 * (pi + 1)] = pat

    xT_b = np.ascontiguousarray(x.reshape(S, DIM).T).astype(BF)
    wo_b = np.ascontiguousarray(wo[_wo_perm_rows(), :]).astype(BF)

    in_maps = []
    for c in range(N_CORES):
        wq_c = np.empty((DIM, NREP * HD), np.float32)
        for hl in range(NREP):
            h = NREP * c + hl
            cols = HD * h + hperm
            wq_c[:, HD * hl:HD * (hl + 1)] = wq[:, cols] * SCALE
        wkv_c = np.empty((DIM, 2 * HD), np.float32)
        wkv_c[:, :HD] = wk[:, HD * c + hperm]
        wkv_c[:, HD:] = wv[:, HD * c:HD * (c + 1)]
        in_maps.append({
            "xT": xT_b, "wq": np.ascontiguousarray(wq_c).astype(BF),
            "wkv": np.ascontiguousarray(wkv_c).astype(BF), "wo": wo_b,
            "cosx": cosx, "sinx": sinx, "selA": selA.astype(BF),
            "selB": selB.astype(BF), "ident": ident.astype(BF),
            "onesr": onesr, "maskbias": mb.astype(BF),
        })

    global _LAST_IN_MAPS
    _LAST_IN_MAPS = in_maps
    res = run_bass_kernel_spmd(nc, in_maps, list(range(N_CORES)))
    out = np.concatenate([res.results[c]["out"] for c in range(N_CORES)],
                         axis=0)
    return out.reshape(B, S, DIM).astype(np.float32, copy=False)
